# revision 38
# baseline (speedup 1.0000x reference)
"""HGNN layer (hypergraph message passing) Trainium2 kernel, 8 NeuronCores.

Sharding: one graph per PAIR of cores; within a pair each core owns half the
hyperedge (Ec) / node (Nc) range. Host pre-casts the big matrices: the 0/1
incident matrix H ships as fp8e4 (exact) in the three tiled layouts the PE
streams (hcol k-major, htr quarter-major, hrow chunk-major); Dv/De ship as
fp8e3 scaled by 64 (descale folded into later evacuations); x ships bf16 in
the block-transposed stationary layout. All streams ride the ACT HWDGE queue
as 1 MB slab DMAs; bounce buffers use the SP queue. Dataflow computes
hxx = H^T x first, then hx = hxx W + b (x) dege (bias as a rank-1 PE
accumulate against a host-computed edge-degree row); attention scores via an
exact fp32 theta matmul on hxx. Softmax is unnormalized; z rides the first
collective as a bf16 hi/lo pair; 1/z and the Dv descale fold into the h1c
evacuation. Cross-core reductions: the two mid-chain exchanges are each two
half-width bf16 AllGathers with a local DVE add (payloads pre-transposed
into the consumer's stationary layout, so the second half overlaps the
first half's consumers and there is no post-collective rearrangement); the
final reduction is two ReduceScatters interleaved with S11's matmul
quarter-pairs, each core keeping only its own node-half of the output.
Dependency-gated dummy collectives re-sync the pair and keep ncfw warm just
before each real collective cluster."""

import numpy as np

B, N, E, D = 4, 4096, 4096, 128
HALF = N // 2
NCORES = 8
PAIRS = [[0, 1], [2, 3], [4, 5], [6, 7]]
BN_EPS = 1e-5
F = 512                 # moving free-dim per matmul
NT = N // 128           # 32 k-tiles over a full 4096 dim
HT = HALF // 128        # 16 k-tiles over a half
SLAB = 4                # k-tiles per stream DMA (1 MB slabs)
DSCALE = 64.0           # host-side scale on Dv/De before fp8e3 cast
ZPAD = 16               # extra bf16 cols on the first AR chunk for z hi/lo

_CACHE = {}


def _build():
    import concourse.bacc as bacc
    import concourse.mybir as mybir
    import concourse.tile as tile
    from concourse.masks import make_identity
    from contextlib import ExitStack

    fp32 = mybir.dt.float32
    bf16 = mybir.dt.bfloat16
    fp8 = mybir.dt.float8e4
    fp8d = mybir.dt.float8e3
    Act = mybir.ActivationFunctionType
    Alu = mybir.AluOpType

    nc = bacc.Bacc("TRN2", target_bir_lowering=False, debug=False,
                   num_devices=NCORES)

    xv_d = nc.dram_tensor("xv", [128, N], bf16, kind="ExternalInput")
    hcol_d = nc.dram_tensor("hcol", [128, NT * HALF], fp8, kind="ExternalInput")
    # htr in quarter-major tiled layout: [p, q*(HT*Q) + t*Q + e], quarter q
    # covers output cols q*1024:(q+1)*1024
    htr_d = nc.dram_tensor("htr", [128, 2 * HT * HALF], fp8,
                           kind="ExternalInput")
    hrow_d = nc.dram_tensor("hrow", [128, 2 * HT * HALF], fp8,
                            kind="ExternalInput")
    dvT_d = nc.dram_tensor("dvT", [128, NT * HALF], fp8d, kind="ExternalInput")
    deT_d = nc.dram_tensor("deT", [128, NT * HALF], fp8d, kind="ExternalInput")
    dege_d = nc.dram_tensor("dege", [1, HALF], fp32, kind="ExternalInput")
    whi_d = nc.dram_tensor("whi", [D, D], bf16, kind="ExternalInput")
    th2_d = nc.dram_tensor("th2", [D, 1], fp32, kind="ExternalInput")
    brow_d = nc.dram_tensor("brow", [1, D], fp32, kind="ExternalInput")
    mask_d = nc.dram_tensor("mask", [1, HALF], fp32, kind="ExternalInput")
    eps_d = nc.dram_tensor("eps", [D, 1], fp32, kind="ExternalInput")
    bng_d = nc.dram_tensor("bng", [D, 1], fp32, kind="ExternalInput")
    bnb_d = nc.dram_tensor("bnb", [D, 1], fp32, kind="ExternalInput")
    bnm_d = nc.dram_tensor("bnm", [D, 1], fp32, kind="ExternalInput")
    bnv_d = nc.dram_tensor("bnv", [D, 1], fp32, kind="ExternalInput")
    y_d = nc.dram_tensor("y", [D, HALF], fp32, kind="ExternalOutput")

    with tile.TileContext(nc) as tc, ExitStack() as ctx:
        const = ctx.enter_context(tc.tile_pool(name="const", bufs=1))
        stream = ctx.enter_context(tc.tile_pool(name="stream", bufs=4))
        qstream_pool = ctx.enter_context(tc.tile_pool(name="qstream", bufs=6))
        med = ctx.enter_context(tc.tile_pool(name="med", bufs=1))
        small = ctx.enter_context(tc.tile_pool(name="small", bufs=1))
        ps = ctx.enter_context(tc.tile_pool(name="ps", bufs=8, space="PSUM"))
        dram = ctx.enter_context(tc.tile_pool(name="dram", bufs=1, space="DRAM"))

        ident = const.tile([128, 128], fp32)
        make_identity(nc, ident)
        one11 = const.tile([1, 1], fp32)
        nc.vector.memset(one11[:], 1.0)
        ones_row = const.tile([1, 128], fp32)
        nc.vector.memset(ones_row[:], 1.0)
        c64 = const.tile([128, 1], fp32)
        nc.vector.memset(c64[:], 1.0 / DSCALE)

        xv = const.tile([128, N], bf16)
        nc.sync.dma_start(out=xv[:], in_=xv_d.ap())

        def load_param(dt_):
            t = const.tile([D, 1], fp32, tag=dt_.name + "_p")
            nc.sync.dma_start(out=t[:], in_=dt_.ap())
            return t

        whi_t = const.tile([D, D], bf16)
        nc.sync.dma_start(out=whi_t[:], in_=whi_d.ap())
        thf_t = const.tile([D, 1], fp32)
        nc.sync.dma_start(out=thf_t[:], in_=th2_d.ap())
        brow_t = const.tile([1, D], fp32)
        nc.sync.dma_start(out=brow_t[:], in_=brow_d.ap())
        dege_t = const.tile([1, HALF], fp32)
        nc.sync.dma_start(out=dege_t[:], in_=dege_d.ap())
        eps_t = load_param(eps_d)
        bng_t = load_param(bng_d)
        bnb_t = load_param(bnb_d)
        bnm_t = load_param(bnm_d)
        bnv_t = load_param(bnv_d)
        mask_t = const.tile([1, HALF], fp32)
        nc.sync.dma_start(out=mask_t[:], in_=mask_d.ap())

        # dummy collectives keep ncfw warm between real collective clusters
        # (a cold cc stream adds ~10us to the next collective)
        warm_n = [0]

        def warm_cc(dep=None):
            wi = dram.tile([1, 16], bf16, tag=f"warmi{warm_n[0]}",
                           name=f"warmi{warm_n[0]}")
            wo = dram.tile([2, 16], bf16, tag=f"warmo{warm_n[0]}",
                           name=f"warmo{warm_n[0]}")
            warm_n[0] += 1
            if dep is not None:
                nc.sync.dma_start(out=wi[:], in_=dep)
            nc.gpsimd.collective_compute(
                "AllGather", Alu.bypass, replica_groups=PAIRS,
                ins=[wi.opt()], outs=[wo.opt()])

        warm_cc()

        # htr quarter-slab stream: quarter q (1024 output cols), 4 t-tiles
        # per 1 MB slab, on the ACT queue
        def make_qstate():
            return {}

        def qtile(state, t, q, name):
            Q4 = HALF // 2
            key = (q, t // SLAB)
            if key not in state:
                sb = qstream_pool.tile([128, SLAB * Q4], fp8, tag="qslab",
                                       name=name)
                base = q * (HT * Q4) + (t // SLAB) * SLAB * Q4
                nc.scalar.dma_start(
                    out=sb[:], in_=htr_d.ap()[:, base:base + SLAB * Q4])
                state[key] = sb
            return state[key][:, (t % SLAB) * Q4:(t % SLAB + 1) * Q4]

        def slab_stream(dram_t, dt, n_tiles, name, gates=None):
            """Yield (k_tile_index, moving_tile_fn) streaming 1MB slabs.
            gates: optional {slab_idx: BassInstruction} — slabs at/after a
            gate index wait on it, so their HBM traffic does not contend
            with the collective they transitively depend on."""
            for s in range(n_tiles // SLAB):
                sb = stream.tile([128, SLAB * HALF], dt, tag="slab",
                                 name=name)
                h = nc.scalar.dma_start(
                    out=sb[:],
                    in_=dram_t.ap()[:, s * SLAB * HALF:(s + 1) * SLAB * HALF])
                if gates:
                    gk = [k for k in gates if k <= s]
                    if gk:
                        tile.add_dep_helper(h.ins, gates[max(gk)].ins,
                                            reason="slab gated on collective")
                for jj in range(SLAB):
                    j = s * SLAB + jj
                    yield j, sb[:, jj * HALF:(jj + 1) * HALF]

        # ---- S2: hxxT [D, HALF] = (H[:,Ec]^T x)^T ------------------------
        hxx_ps = [ps.tile([128, F], fp32, tag="ps", name=f"hxx{i}")
                  for i in range(HALF // F)]
        for j, hj in slab_stream(hcol_d, fp8, NT, "hj"):
            for blk in range(HALF // F):
                nc.tensor.matmul(hxx_ps[blk][:],
                                 xv[:, j * D:(j + 1) * D],
                                 hj[:, blk * F:(blk + 1) * F],
                                 start=(j == 0), stop=(j == NT - 1))
        hxxT = med.tile([D, HALF], fp32, tag="hxxT")
        hxx_hi = med.tile([D, HALF], bf16, tag="hxx_hi")
        for blk in range(HALF // F):
            sl = slice(blk * F, (blk + 1) * F)
            nc.vector.tensor_copy(hxxT[:, sl], hxx_ps[blk][:])
            nc.vector.tensor_copy(hxx_hi[:, sl], hxx_ps[blk][:])

        # ---- S3: hxT = W^T hxx + b (x) dege ; st = th^T hxx (fp32) -------
        hxT = med.tile([D, HALF], fp32, tag="hxT")
        st_sb = small.tile([1, HALF], fp32, tag="st_sb")
        for blk in range(HALF // F):
            sl = slice(blk * F, (blk + 1) * F)
            hx2 = ps.tile([128, F], fp32, tag="ps", name=f"hx2_{blk}")
            nc.tensor.matmul(hx2[:], whi_t[:], hxx_hi[:, sl],
                             start=True, stop=False)
            nc.tensor.matmul(hx2[:], brow_t[:], dege_t[:, sl],
                             start=False, stop=True)
            nc.vector.tensor_copy(hxT[:, sl], hx2[:])
            sp = ps.tile([1, F], fp32, tag="ps", name=f"sp{blk}")
            nc.tensor.matmul(sp[:], thf_t[:], hxxT[:, sl],
                             start=True, stop=True)
            nc.vector.tensor_copy(st_sb[:, sl], sp[:])

        # ---- S4: softmax pieces (in-place on st_sb) ----------------------
        attn_u = st_sb
        nc.scalar.activation(attn_u[:], st_sb[:], Act.Exp)
        nc.vector.tensor_mul(attn_u[:], attn_u[:], mask_t[:])
        z_t = small.tile([1, 1], fp32, tag="z_t")
        nc.vector.reduce_sum(z_t[:], attn_u[:], axis=mybir.AxisListType.X)
        # z hi/lo bf16 pieces
        zhi = small.tile([1, 1], bf16, tag="zhi")
        zlo = small.tile([1, 1], bf16, tag="zlo")
        zf = small.tile([1, 1], fp32, tag="zf")
        nc.vector.tensor_copy(zhi[:], z_t[:])
        nc.vector.tensor_copy(zf[:], zhi[:])
        nc.vector.tensor_tensor(zf[:], z_t[:], zf[:], op=Alu.subtract)
        nc.vector.tensor_copy(zlo[:], zf[:])
        attnv = med.tile([128, HT], fp32, tag="attnv")
        for t in range(HT):
            pt = ps.tile([128, 1], fp32, tag="ps")
            nc.tensor.matmul(pt[:], attn_u[:, t * 128:(t + 1) * 128], one11[:],
                             start=True, stop=True)
            nc.vector.tensor_copy(attnv[:, t:t + 1], pt[:])
        ehxT = med.tile([D, HALF], fp32, tag="ehxT")
        nc.vector.tensor_scalar_mul(ehxT[:], hxT[:], eps_t[:])

        # ---- S5: h1av [128, HT*D] bf16 = attn * hx (e-part tiles) --------
        h1av = med.tile([128, HALF], bf16, tag="h1av")
        for t in range(HT):
            pt = ps.tile([128, 128], fp32, tag="ps")
            nc.tensor.transpose(pt[:], hxT[:, t * 128:(t + 1) * 128], ident[:])
            nc.vector.tensor_scalar_mul(h1av[:, t * 128:(t + 1) * 128], pt[:],
                                        attnv[:, t:t + 1])

        def chunked_bmm_ag_v(stationary, moving_of, tagbase, with_z=False):
            """Two half-width partial bmms; payload pre-transposed to the
            consumer's v-layout, cast bf16, AllGathered per chunk with a
            local DVE add of the two rank blocks (cheaper than ncfw
            AllReduce). Returns (resA, resB, ccA_inst, ccB_inst)."""
            outs = []
            cc_insts = []
            for chunk in range(2):
                w = HALF + ZPAD if (with_z and chunk == 0) else HALF
                pss = [ps.tile([128, F], fp32, tag="ps",
                               name=f"{tagbase}_{chunk}_{i}")
                       for i in range(HALF // F)]
                for t in range(HT):
                    for blk in range(HALF // F):
                        nc.tensor.matmul(
                            pss[blk][:],
                            stationary[:, t * 128:(t + 1) * 128],
                            moving_of(t, chunk * HALF + blk * F, F),
                            start=(t == 0), stop=(t == HT - 1))
                ccT = med.tile([D, HALF], fp32, tag="ccsbT")
                for blk in range(HALF // F):
                    sl = slice(blk * F, (blk + 1) * F)
                    nc.vector.tensor_copy(ccT[:, sl], pss[blk][:])
                ccv = med.tile([128, HALF + ZPAD], bf16,
                               tag=f"ccv{chunk}")
                for t in range(HT):
                    pt = ps.tile([128, 128], fp32, tag="ps")
                    nc.tensor.transpose(pt[:], ccT[:, t * 128:(t + 1) * 128],
                                        ident[:])
                    nc.vector.tensor_copy(ccv[:, t * 128:(t + 1) * 128], pt[:])
                if with_z and chunk == 0:
                    nc.vector.memset(ccv[:, HALF:], 0.0)
                    nc.vector.tensor_copy(ccv[0:1, HALF:HALF + 1], zhi[:])
                    nc.vector.tensor_copy(ccv[0:1, HALF + 1:HALF + 2], zlo[:])
                cc_in = dram.tile([128, w], bf16, tag=f"{tagbase}i{chunk}")
                cc_out = dram.tile([256, w], bf16, tag=f"{tagbase}o{chunk}")
                nc.sync.dma_start(out=cc_in[:], in_=ccv[:, 0:w])
                cc_h = nc.gpsimd.collective_compute(
                    "AllGather", Alu.bypass, replica_groups=PAIRS,
                    ins=[cc_in.opt()], outs=[cc_out.opt()])
                cc_insts.append(cc_h)
                res = med.tile([128, HALF + ZPAD], bf16,
                               tag=f"resv{chunk}")
                agt = med.tile([128, HALF + ZPAD], bf16, tag="agtmp")
                nc.sync.dma_start(out=res[:, 0:w], in_=cc_out[0:128, :])
                nc.sync.dma_start(out=agt[:, 0:w], in_=cc_out[128:256, :])
                nc.vector.tensor_tensor(res[:, 0:w], res[:, 0:w],
                                        agt[:, 0:w], op=Alu.add)
                outs.append(res)
            return outs + cc_insts

        # ---- S6: h1b = H h1a (partial over Ec), v-layout chunked AG ------
        htr_s6 = make_qstate()

        def htr_moving(t, lo, w):
            q, off = divmod(lo, HALF // 2)
            return qtile(htr_s6, t, q, "htq6")[:, off:off + w]

        warm_cc(dep=h1av[0:1, 0:16])
        h1bA, h1bB, cc1A, cc1B = chunked_bmm_ag_v(h1av, htr_moving, "cc1",
                                                  with_z=True)

        # rz = 1/(z), folded with 1/DSCALE, broadcast to [128, 1]
        rz = small.tile([1, 1], fp32, tag="rz")
        zs = small.tile([1, 1], fp32, tag="zs")
        nc.vector.tensor_copy(rz[:], h1bA[0:1, HALF:HALF + 1])
        nc.vector.tensor_copy(zs[:], h1bA[0:1, HALF + 1:HALF + 2])
        nc.vector.tensor_tensor(rz[:], rz[:], zs[:], op=Alu.add)
        nc.vector.reciprocal(rz[:], rz[:])
        rz_ps = ps.tile([128, 1], fp32, tag="ps")
        nc.tensor.matmul(rz_ps[:], ones_row[:], rz[:], start=True, stop=True)
        rz_bc = small.tile([128, 1], fp32, tag="rz_bc")
        nc.vector.tensor_copy(rz_bc[:], rz_ps[:])
        nc.vector.tensor_mul(rz_bc[:], rz_bc[:], c64[:])

        def vtile(resA, resB, j):
            src = resA if j < HT else resB
            jj = j % HT
            return src[:, jj * 128:(jj + 1) * 128]

        # ---- S7: h1cT = (Dv[Nc,:] h1b)^T * rz/DSCALE ---------------------
        h1c_ps = [ps.tile([128, F], fp32, tag="ps", name=f"h1c{i}")
                  for i in range(HALF // F)]
        for j, dj in slab_stream(dvT_d, fp8d, NT, "dj",
                                 gates={2: cc1A, 4: cc1B}):
            for blk in range(HALF // F):
                nc.tensor.matmul(h1c_ps[blk][:], vtile(h1bA, h1bB, j),
                                 dj[:, blk * F:(blk + 1) * F],
                                 start=(j == 0), stop=(j == NT - 1))
        h1cT = med.tile([D, HALF], fp32, tag="hxxT")
        for blk in range(HALF // F):
            sl = slice(blk * F, (blk + 1) * F)
            nc.vector.tensor_scalar_mul(h1cT[:, sl], h1c_ps[blk][:], rz_bc[:])
        h1cv = med.tile([128, HALF], bf16, tag="h1cv")
        for t in range(HT):
            pt = ps.tile([128, 128], fp32, tag="ps")
            nc.tensor.transpose(pt[:], h1cT[:, t * 128:(t + 1) * 128], ident[:])
            nc.vector.tensor_copy(h1cv[:, t * 128:(t + 1) * 128], pt[:])
        warm_cc(dep=h1cv[0:1, 0:16])

        # ---- S8: h1d = H[Nc,:]^T h1c (partial over Nc), chunked AR -------
        hrow_state = {}

        def hrow_moving(t, lo, w):
            chunk = lo // HALF
            slab_i = t // SLAB
            key = (chunk, slab_i)
            if key not in hrow_state:
                sb = stream.tile([128, SLAB * HALF], fp8, tag="slab",
                                 name=f"rj{chunk}")
                base = chunk * HT * HALF + slab_i * SLAB * HALF
                nc.scalar.dma_start(
                    out=sb[:], in_=hrow_d.ap()[:, base:base + SLAB * HALF])
                hrow_state[key] = sb
            return hrow_state[key][:, (t % SLAB) * HALF + (lo % HALF):
                                   (t % SLAB) * HALF + (lo % HALF) + w]

        h1dA, h1dB, cc2A, cc2B = chunked_bmm_ag_v(h1cv, hrow_moving, "cc2")

        # ---- S9+S10: hT = (De[Ec,:] h1d)^T / DSCALE + eps*hx -------------
        h1e_ps = [ps.tile([128, F], fp32, tag="ps", name=f"h1e{i}")
                  for i in range(HALF // F)]
        for j, ej in slab_stream(deT_d, fp8d, NT, "ej",
                                 gates={2: cc2A, 4: cc2B}):
            for blk in range(HALF // F):
                nc.tensor.matmul(h1e_ps[blk][:], vtile(h1dA, h1dB, j),
                                 ej[:, blk * F:(blk + 1) * F],
                                 start=(j == 0), stop=(j == NT - 1))
        hT = med.tile([D, HALF], fp32, tag="hxT")
        for blk in range(HALF // F):
            sl = slice(blk * F, (blk + 1) * F)
            nc.vector.scalar_tensor_tensor(hT[:, sl], h1e_ps[blk][:],
                                           1.0 / DSCALE, ehxT[:, sl],
                                           op0=Alu.mult, op1=Alu.add)
        hv = med.tile([128, HALF], bf16, tag="h1av")
        for t in range(HT):
            pt = ps.tile([128, 128], fp32, tag="ps")
            nc.tensor.transpose(pt[:], hT[:, t * 128:(t + 1) * 128], ident[:])
            nc.vector.tensor_copy(hv[:, t * 128:(t + 1) * 128], pt[:])
        warm_cc(dep=hv[0:1, 0:16])

        # ---- S11: out = H h (partial over Ec), bf16 T-layout chunked AR --
        s_bn = small.tile([D, 1], fp32, tag="s_bn")
        nc.vector.tensor_scalar_add(s_bn[:], bnv_t[:], BN_EPS)
        nc.scalar.activation(s_bn[:], s_bn[:], Act.Sqrt)
        nc.vector.reciprocal(s_bn[:], s_bn[:])
        nc.vector.tensor_mul(s_bn[:], s_bn[:], bng_t[:])
        t_bn = small.tile([D, 1], fp32, tag="t_bn")
        nc.vector.tensor_mul(t_bn[:], bnm_t[:], s_bn[:])
        nc.vector.tensor_tensor(t_bn[:], bnb_t[:], t_bn[:], op=Alu.subtract)

        # two half-width ReduceScatters (core even gets summed cols 0:HALF,
        # odd the rest). S11's matmuls are grouped into quarter-pairs so
        # RS h=0 fires after only half the matmuls; its epilogue overlaps
        # the rest of S11 and RS h=1.
        Q = HALF // 2
        cc3_in = [dram.tile([256, Q], bf16, tag=f"cc3i{h}", name=f"cc3i{h}")
                  for h in range(2)]
        cc3_out = [dram.tile([128, Q], bf16, tag=f"cc3o{h}", name=f"cc3o{h}")
                   for h in range(2)]
        htr_s11 = make_qstate()
        for h in range(2):
            # blocks covering cols [h*Q:(h+1)*Q] of both n-chunks
            pss = [ps.tile([128, F], fp32, tag="ps", name=f"out_{h}_{i}")
                   for i in range(4)]
            for t in range(HT):
                for i in range(4):
                    chunk, blk = divmod(i, 2)
                    q = 2 * chunk + h
                    nc.tensor.matmul(
                        pss[i][:],
                        hv[:, t * 128:(t + 1) * 128],
                        qtile(htr_s11, t, q, "htq11")[:, blk * F:
                                                      (blk + 1) * F],
                        start=(t == 0), stop=(t == HT - 1))
            ccv = med.tile([128, HALF + ZPAD], bf16, tag=f"ccv{h}")
            for i in range(4):
                chunk, blk = divmod(i, 2)
                sl = slice(chunk * Q + blk * F, chunk * Q + (blk + 1) * F)
                nc.vector.tensor_copy(ccv[:, sl], pss[i][:])
            for chunk in range(2):
                nc.sync.dma_start(
                    out=cc3_in[h][chunk * 128:(chunk + 1) * 128, :],
                    in_=ccv[:, chunk * Q:(chunk + 1) * Q])
            nc.gpsimd.collective_compute(
                "ReduceScatter", Alu.add, replica_groups=PAIRS,
                ins=[cc3_in[h].opt()], outs=[cc3_out[h].opt()])
        for h in range(2):
            res3 = med.tile([128, HALF + ZPAD], bf16, tag=f"resv{h}")
            nc.sync.dma_start(out=res3[:, 0:Q], in_=cc3_out[h][:])
            of = med.tile([D, Q], fp32, tag=f"of{h}")
            nc.scalar.activation(of[:], res3[:, 0:Q], Act.Lrelu, alpha=0.01)
            nc.vector.tensor_scalar(of[:], of[:], s_bn[:], t_bn[:],
                                    op0=Alu.mult, op1=Alu.add)
            nc.sync.dma_start(out=y_d.ap()[:, h * Q:(h + 1) * Q], in_=of[:])

    nc.finalize()
    return nc


def _get_nc():
    if "nc" not in _CACHE:
        _CACHE["nc"] = _build()
    return _CACHE["nc"]


def _tile128(a):
    """[K*128, W] -> [128, K*W] block-transposed stream layout."""
    K = a.shape[0] // 128
    return np.ascontiguousarray(
        a.reshape(K, 128, a.shape[1]).transpose(1, 0, 2).reshape(
            128, K * a.shape[1]))


def _shard(inputs):
    import ml_dtypes
    bf16 = ml_dtypes.bfloat16
    fp8 = ml_dtypes.float8_e4m3
    fp8d = ml_dtypes.float8_e3m4

    H = np.asarray(inputs["incident_mat"], dtype=np.float32)
    Dv = np.asarray(inputs["degree_v"], dtype=np.float32)
    De = np.asarray(inputs["degree_e"], dtype=np.float32)
    x = np.asarray(inputs["x"], dtype=np.float32)
    em = np.asarray(inputs["e_masks"])
    w = np.asarray(inputs["mlp_W"], dtype=np.float32)
    b = np.asarray(inputs["mlp_b"], dtype=np.float32)
    th = np.asarray(inputs["theta_att"], dtype=np.float32).reshape(D)
    eps = np.full((D, 1), float(np.asarray(inputs["eps"]).reshape(-1)[0]),
                  dtype=np.float32)

    def col(v):
        return np.ascontiguousarray(
            np.asarray(v, dtype=np.float32).reshape(D, 1))

    bng, bnb = col(inputs["bn_gamma"]), col(inputs["bn_beta"])
    bnm, bnv = col(inputs["bn_mean"]), col(inputs["bn_var"])

    whi = w.astype(bf16)
    th2 = np.ascontiguousarray(th.reshape(D, 1))
    brow = np.ascontiguousarray(b.reshape(1, D))

    in_maps = []
    for g in range(B):
        Hg8 = H[g].astype(fp8)
        HgT8 = np.ascontiguousarray(H[g].T).astype(fp8)
        dege_full = H[g].sum(axis=0, dtype=np.float32)
        xv = _tile128(x[g]).astype(bf16)
        DvT = (Dv[g].T * DSCALE).astype(fp8d)
        DeT = (De[g].T * DSCALE).astype(fp8d)
        for c in range(2):
            lo, hi = c * HALF, (c + 1) * HALF
            # hrow chunk-major: [128, 2*HT*HALF], chunk c2 at t*HALF+e
            hrow_g = Hg8[lo:hi, :].reshape(HT, 128, 2, HALF)
            hrow_t = np.ascontiguousarray(
                hrow_g.transpose(1, 2, 0, 3).reshape(128, 2 * HT * HALF))
            htr_q = np.ascontiguousarray(
                HgT8[lo:hi, :].reshape(HT, 128, 4, HALF // 2)
                .transpose(1, 2, 0, 3).reshape(128, 2 * HT * HALF))
            in_maps.append({
                "xv": xv,
                "hcol": _tile128(np.ascontiguousarray(Hg8[:, lo:hi])),
                "htr": htr_q,
                "hrow": hrow_t,
                "dvT": _tile128(np.ascontiguousarray(DvT[:, lo:hi])),
                "deT": _tile128(np.ascontiguousarray(DeT[:, lo:hi])),
                "dege": np.ascontiguousarray(
                    dege_full[lo:hi].reshape(1, HALF)),
                "whi": whi, "th2": th2, "brow": brow,
                "mask": np.ascontiguousarray(
                    em[g, lo:hi].astype(np.float32).reshape(1, HALF)),
                "eps": eps,
                "bng": bng, "bnb": bnb, "bnm": bnm, "bnv": bnv,
            })
    return in_maps


def kernel(**inputs):
    from concourse.bass_utils import run_bass_kernel_spmd

    nc = _get_nc()
    in_maps = _shard(inputs)
    res = run_bass_kernel_spmd(nc, in_maps, list(range(NCORES)))
    out = np.empty((B, N, D), dtype=np.float32)
    for g in range(B):
        out[g, :HALF, :] = res.results[2 * g]["y"].T
        out[g, HALF:, :] = res.results[2 * g + 1]["y"].T
    return out


# revision 39
# speedup vs baseline: 1.0397x; 1.0397x over previous
"""HGNN layer (hypergraph message passing) Trainium2 kernel, 8 NeuronCores.

Sharding: one graph per PAIR of cores; within a pair each core owns half the
hyperedge (Ec) / node (Nc) range. Host pre-casts the big matrices: the 0/1
incident matrix H ships as fp8e4 (exact) in the three layouts the PE needs;
Dv/De ship as fp8e3 scaled by 64 (descale folded into later evacuations); x
ships bf16 in block-transposed stationary layout. Streams use host-tiled
[128, k*HALF] layouts so slab DMAs move 1 MB at a time on the ACT HWDGE
queue. Dataflow computes hxx = H^T x first, then hx = hxx W + b (x) dege
(bias as a rank-1 PE accumulate against a host-computed edge-degree row);
attention scores use a hi/lo bf16 split of hxx and theta. Softmax is
unnormalized; z rides the first AllReduce as a bf16 hi/lo pair; 1/z and the
Dv descale fold into the h1c evacuation. htr stays SBUF-resident for its two
uses. Each of the 3 pair-AllReduces is split into two half-width bf16
collectives whose payloads are pre-transposed into the consumer's layout, so
the second half overlaps the first half's consumers and there is zero
post-AR rearrangement."""

import numpy as np

B, N, E, D = 4, 4096, 4096, 128
HALF = N // 2
NCORES = 8
PAIRS = [[0, 1], [2, 3], [4, 5], [6, 7]]
BN_EPS = 1e-5
F = 512                 # moving free-dim per matmul
NT = N // 128           # 32 k-tiles over a full 4096 dim
HT = HALF // 128        # 16 k-tiles over a half
SLAB = 4                # k-tiles per stream DMA (1 MB slabs)
DSCALE = 64.0           # host-side scale on Dv/De before fp8e3 cast
ZPAD = 16               # extra bf16 cols on the first AR chunk for z hi/lo

_CACHE = {}


def _build():
    import concourse.bacc as bacc
    import concourse.mybir as mybir
    import concourse.tile as tile
    from concourse.masks import make_identity
    from contextlib import ExitStack

    fp32 = mybir.dt.float32
    bf16 = mybir.dt.bfloat16
    fp8 = mybir.dt.float8e4
    fp8d = mybir.dt.float8e3
    Act = mybir.ActivationFunctionType
    Alu = mybir.AluOpType

    nc = bacc.Bacc("TRN2", target_bir_lowering=False, debug=False,
                   num_devices=NCORES)

    xv_d = nc.dram_tensor("xv", [128, N], bf16, kind="ExternalInput")
    hcol_d = nc.dram_tensor("hcol", [128, NT * HALF], fp8, kind="ExternalInput")
    # htr in quarter-major tiled layout: [p, q*(HT*Q) + t*Q + e], quarter q
    # covers output cols q*1024:(q+1)*1024
    htr_d = nc.dram_tensor("htr", [128, 2 * HT * HALF], fp8,
                           kind="ExternalInput")
    hrow_d = nc.dram_tensor("hrow", [128, 2 * HT * HALF], fp8,
                            kind="ExternalInput")
    dvT_d = nc.dram_tensor("dvT", [128, NT * HALF], fp8d, kind="ExternalInput")
    deT_d = nc.dram_tensor("deT", [128, NT * HALF], fp8d, kind="ExternalInput")
    dege_d = nc.dram_tensor("dege", [1, HALF], fp32, kind="ExternalInput")
    whi_d = nc.dram_tensor("whi", [D, D], bf16, kind="ExternalInput")
    th2_d = nc.dram_tensor("th2", [D, 1], fp32, kind="ExternalInput")
    brow_d = nc.dram_tensor("brow", [1, D], fp32, kind="ExternalInput")
    mask_d = nc.dram_tensor("mask", [1, HALF], fp32, kind="ExternalInput")
    eps_d = nc.dram_tensor("eps", [D, 1], fp32, kind="ExternalInput")
    bng_d = nc.dram_tensor("bng", [D, 1], fp32, kind="ExternalInput")
    bnb_d = nc.dram_tensor("bnb", [D, 1], fp32, kind="ExternalInput")
    bnm_d = nc.dram_tensor("bnm", [D, 1], fp32, kind="ExternalInput")
    bnv_d = nc.dram_tensor("bnv", [D, 1], fp32, kind="ExternalInput")
    y_d = nc.dram_tensor("y", [D, HALF], fp32, kind="ExternalOutput")

    with tile.TileContext(nc) as tc, ExitStack() as ctx:
        const = ctx.enter_context(tc.tile_pool(name="const", bufs=1))
        stream = ctx.enter_context(tc.tile_pool(name="stream", bufs=4))
        qstream_pool = ctx.enter_context(tc.tile_pool(name="qstream", bufs=6))
        med = ctx.enter_context(tc.tile_pool(name="med", bufs=1))
        small = ctx.enter_context(tc.tile_pool(name="small", bufs=1))
        ps = ctx.enter_context(tc.tile_pool(name="ps", bufs=8, space="PSUM"))
        dram = ctx.enter_context(tc.tile_pool(name="dram", bufs=1, space="DRAM"))

        ident = const.tile([128, 128], fp32)
        make_identity(nc, ident)
        one11 = const.tile([1, 1], fp32)
        nc.vector.memset(one11[:], 1.0)
        ones_row = const.tile([1, 128], fp32)
        nc.vector.memset(ones_row[:], 1.0)
        c64 = const.tile([128, 1], fp32)
        nc.vector.memset(c64[:], 1.0 / DSCALE)

        xv = const.tile([128, N], bf16)
        nc.sync.dma_start(out=xv[:], in_=xv_d.ap())

        def load_param(dt_):
            t = const.tile([D, 1], fp32, tag=dt_.name + "_p")
            nc.sync.dma_start(out=t[:], in_=dt_.ap())
            return t

        whi_t = const.tile([D, D], bf16)
        nc.sync.dma_start(out=whi_t[:], in_=whi_d.ap())
        thf_t = const.tile([D, 1], fp32)
        nc.sync.dma_start(out=thf_t[:], in_=th2_d.ap())
        brow_t = const.tile([1, D], fp32)
        nc.sync.dma_start(out=brow_t[:], in_=brow_d.ap())
        dege_t = const.tile([1, HALF], fp32)
        nc.sync.dma_start(out=dege_t[:], in_=dege_d.ap())
        eps_t = load_param(eps_d)
        bng_t = load_param(bng_d)
        bnb_t = load_param(bnb_d)
        bnm_t = load_param(bnm_d)
        bnv_t = load_param(bnv_d)
        mask_t = const.tile([1, HALF], fp32)
        nc.sync.dma_start(out=mask_t[:], in_=mask_d.ap())

        # dummy collectives keep ncfw warm between real collective clusters
        # (a cold cc stream adds ~10us to the next collective)
        warm_n = [0]

        def warm_cc():
            wi = dram.tile([1, 16], bf16, tag=f"warmi{warm_n[0]}",
                           name=f"warmi{warm_n[0]}")
            wo = dram.tile([2, 16], bf16, tag=f"warmo{warm_n[0]}",
                           name=f"warmo{warm_n[0]}")
            warm_n[0] += 1
            nc.gpsimd.collective_compute(
                "AllGather", Alu.bypass, replica_groups=PAIRS,
                ins=[wi.opt()], outs=[wo.opt()])

        warm_cc()

        # htr quarter-slab stream: quarter q (1024 output cols), 4 t-tiles
        # per 1 MB slab, on the ACT queue
        def make_qstate():
            return {}

        def qtile(state, t, q, name):
            Q4 = HALF // 2
            key = (q, t // SLAB)
            if key not in state:
                sb = qstream_pool.tile([128, SLAB * Q4], fp8, tag="qslab",
                                       name=name)
                base = q * (HT * Q4) + (t // SLAB) * SLAB * Q4
                nc.scalar.dma_start(
                    out=sb[:], in_=htr_d.ap()[:, base:base + SLAB * Q4])
                state[key] = sb
            return state[key][:, (t % SLAB) * Q4:(t % SLAB + 1) * Q4]

        def slab_stream(dram_t, dt, n_tiles, name):
            """Yield (k_tile_index, moving_tile_fn) streaming 1MB slabs."""
            for s in range(n_tiles // SLAB):
                sb = stream.tile([128, SLAB * HALF], dt, tag="slab",
                                 name=name)
                nc.scalar.dma_start(
                    out=sb[:],
                    in_=dram_t.ap()[:, s * SLAB * HALF:(s + 1) * SLAB * HALF])
                for jj in range(SLAB):
                    j = s * SLAB + jj
                    yield j, sb[:, jj * HALF:(jj + 1) * HALF]

        # ---- S2: hxxT [D, HALF] = (H[:,Ec]^T x)^T ------------------------
        hxx_ps = [ps.tile([128, F], fp32, tag="ps", name=f"hxx{i}")
                  for i in range(HALF // F)]
        for j, hj in slab_stream(hcol_d, fp8, NT, "hj"):
            for blk in range(HALF // F):
                nc.tensor.matmul(hxx_ps[blk][:],
                                 xv[:, j * D:(j + 1) * D],
                                 hj[:, blk * F:(blk + 1) * F],
                                 start=(j == 0), stop=(j == NT - 1))
        hxxT = med.tile([D, HALF], fp32, tag="hxxT")
        hxx_hi = med.tile([D, HALF], bf16, tag="hxx_hi")
        for blk in range(HALF // F):
            sl = slice(blk * F, (blk + 1) * F)
            nc.vector.tensor_copy(hxxT[:, sl], hxx_ps[blk][:])
            nc.vector.tensor_copy(hxx_hi[:, sl], hxx_ps[blk][:])

        # ---- S3: hxT = W^T hxx + b (x) dege ; st = th^T hxx (fp32) -------
        hxT = med.tile([D, HALF], fp32, tag="hxT")
        st_sb = small.tile([1, HALF], fp32, tag="st_sb")
        for blk in range(HALF // F):
            sl = slice(blk * F, (blk + 1) * F)
            hx2 = ps.tile([128, F], fp32, tag="ps", name=f"hx2_{blk}")
            nc.tensor.matmul(hx2[:], whi_t[:], hxx_hi[:, sl],
                             start=True, stop=False)
            nc.tensor.matmul(hx2[:], brow_t[:], dege_t[:, sl],
                             start=False, stop=True)
            nc.vector.tensor_copy(hxT[:, sl], hx2[:])
            sp = ps.tile([1, F], fp32, tag="ps", name=f"sp{blk}")
            nc.tensor.matmul(sp[:], thf_t[:], hxxT[:, sl],
                             start=True, stop=True)
            nc.vector.tensor_copy(st_sb[:, sl], sp[:])

        # ---- S4: softmax pieces (in-place on st_sb) ----------------------
        attn_u = st_sb
        nc.scalar.activation(attn_u[:], st_sb[:], Act.Exp)
        nc.vector.tensor_mul(attn_u[:], attn_u[:], mask_t[:])
        z_t = small.tile([1, 1], fp32, tag="z_t")
        nc.vector.reduce_sum(z_t[:], attn_u[:], axis=mybir.AxisListType.X)
        # z hi/lo bf16 pieces
        zhi = small.tile([1, 1], bf16, tag="zhi")
        zlo = small.tile([1, 1], bf16, tag="zlo")
        zf = small.tile([1, 1], fp32, tag="zf")
        nc.vector.tensor_copy(zhi[:], z_t[:])
        nc.vector.tensor_copy(zf[:], zhi[:])
        nc.vector.tensor_tensor(zf[:], z_t[:], zf[:], op=Alu.subtract)
        nc.vector.tensor_copy(zlo[:], zf[:])
        attnv = med.tile([128, HT], fp32, tag="attnv")
        for t in range(HT):
            pt = ps.tile([128, 1], fp32, tag="ps")
            nc.tensor.matmul(pt[:], attn_u[:, t * 128:(t + 1) * 128], one11[:],
                             start=True, stop=True)
            nc.vector.tensor_copy(attnv[:, t:t + 1], pt[:])
        ehxT = med.tile([D, HALF], fp32, tag="ehxT")
        nc.vector.tensor_scalar_mul(ehxT[:], hxT[:], eps_t[:])
        warm_cc()

        # ---- S5: h1av [128, HT*D] bf16 = attn * hx (e-part tiles) --------
        h1av = med.tile([128, HALF], bf16, tag="h1av")
        for t in range(HT):
            pt = ps.tile([128, 128], fp32, tag="ps")
            nc.tensor.transpose(pt[:], hxT[:, t * 128:(t + 1) * 128], ident[:])
            nc.vector.tensor_scalar_mul(h1av[:, t * 128:(t + 1) * 128], pt[:],
                                        attnv[:, t:t + 1])

        def chunked_bmm_ag_v(stationary, moving_of, tagbase, with_z=False):
            """Two half-width partial bmms; payload pre-transposed to the
            consumer's v-layout, cast bf16, AllGathered per chunk with a
            local DVE add of the two rank blocks (cheaper than ncfw
            AllReduce). Returns (resA, resB) bf16 [128, HALF(+ZPAD)]."""
            outs = []
            for chunk in range(2):
                w = HALF + ZPAD if (with_z and chunk == 0) else HALF
                pss = [ps.tile([128, F], fp32, tag="ps",
                               name=f"{tagbase}_{chunk}_{i}")
                       for i in range(HALF // F)]
                for t in range(HT):
                    for blk in range(HALF // F):
                        nc.tensor.matmul(
                            pss[blk][:],
                            stationary[:, t * 128:(t + 1) * 128],
                            moving_of(t, chunk * HALF + blk * F, F),
                            start=(t == 0), stop=(t == HT - 1))
                ccT = med.tile([D, HALF], fp32, tag="ccsbT")
                for blk in range(HALF // F):
                    sl = slice(blk * F, (blk + 1) * F)
                    nc.vector.tensor_copy(ccT[:, sl], pss[blk][:])
                ccv = med.tile([128, HALF + ZPAD], bf16,
                               tag=f"ccv{chunk}")
                for t in range(HT):
                    pt = ps.tile([128, 128], fp32, tag="ps")
                    nc.tensor.transpose(pt[:], ccT[:, t * 128:(t + 1) * 128],
                                        ident[:])
                    nc.vector.tensor_copy(ccv[:, t * 128:(t + 1) * 128], pt[:])
                if with_z and chunk == 0:
                    nc.vector.memset(ccv[:, HALF:], 0.0)
                    nc.vector.tensor_copy(ccv[0:1, HALF:HALF + 1], zhi[:])
                    nc.vector.tensor_copy(ccv[0:1, HALF + 1:HALF + 2], zlo[:])
                cc_in = dram.tile([128, w], bf16, tag=f"{tagbase}i{chunk}")
                cc_out = dram.tile([256, w], bf16, tag=f"{tagbase}o{chunk}")
                nc.sync.dma_start(out=cc_in[:], in_=ccv[:, 0:w])
                nc.gpsimd.collective_compute(
                    "AllGather", Alu.bypass, replica_groups=PAIRS,
                    ins=[cc_in.opt()], outs=[cc_out.opt()])
                res = med.tile([128, HALF + ZPAD], bf16,
                               tag=f"resv{chunk}")
                agt = med.tile([128, HALF + ZPAD], bf16, tag="agtmp")
                nc.sync.dma_start(out=res[:, 0:w], in_=cc_out[0:128, :])
                nc.sync.dma_start(out=agt[:, 0:w], in_=cc_out[128:256, :])
                nc.vector.tensor_tensor(res[:, 0:w], res[:, 0:w],
                                        agt[:, 0:w], op=Alu.add)
                outs.append(res)
            return outs

        # ---- S6: h1b = H h1a (partial over Ec), v-layout chunked AG ------
        htr_s6 = make_qstate()

        def htr_moving(t, lo, w):
            q, off = divmod(lo, HALF // 2)
            return qtile(htr_s6, t, q, "htq6")[:, off:off + w]

        h1bA, h1bB = chunked_bmm_ag_v(h1av, htr_moving, "cc1", with_z=True)

        # rz = 1/(z), folded with 1/DSCALE, broadcast to [128, 1]
        rz = small.tile([1, 1], fp32, tag="rz")
        zs = small.tile([1, 1], fp32, tag="zs")
        nc.vector.tensor_copy(rz[:], h1bA[0:1, HALF:HALF + 1])
        nc.vector.tensor_copy(zs[:], h1bA[0:1, HALF + 1:HALF + 2])
        nc.vector.tensor_tensor(rz[:], rz[:], zs[:], op=Alu.add)
        nc.vector.reciprocal(rz[:], rz[:])
        rz_ps = ps.tile([128, 1], fp32, tag="ps")
        nc.tensor.matmul(rz_ps[:], ones_row[:], rz[:], start=True, stop=True)
        rz_bc = small.tile([128, 1], fp32, tag="rz_bc")
        nc.vector.tensor_copy(rz_bc[:], rz_ps[:])
        nc.vector.tensor_mul(rz_bc[:], rz_bc[:], c64[:])

        def vtile(resA, resB, j):
            src = resA if j < HT else resB
            jj = j % HT
            return src[:, jj * 128:(jj + 1) * 128]

        # ---- S7: h1cT = (Dv[Nc,:] h1b)^T * rz/DSCALE ---------------------
        h1c_ps = [ps.tile([128, F], fp32, tag="ps", name=f"h1c{i}")
                  for i in range(HALF // F)]
        for j, dj in slab_stream(dvT_d, fp8d, NT, "dj"):
            for blk in range(HALF // F):
                nc.tensor.matmul(h1c_ps[blk][:], vtile(h1bA, h1bB, j),
                                 dj[:, blk * F:(blk + 1) * F],
                                 start=(j == 0), stop=(j == NT - 1))
        h1cT = med.tile([D, HALF], fp32, tag="hxxT")
        for blk in range(HALF // F):
            sl = slice(blk * F, (blk + 1) * F)
            nc.vector.tensor_scalar_mul(h1cT[:, sl], h1c_ps[blk][:], rz_bc[:])
        h1cv = med.tile([128, HALF], bf16, tag="h1cv")
        for t in range(HT):
            pt = ps.tile([128, 128], fp32, tag="ps")
            nc.tensor.transpose(pt[:], h1cT[:, t * 128:(t + 1) * 128], ident[:])
            nc.vector.tensor_copy(h1cv[:, t * 128:(t + 1) * 128], pt[:])
        warm_cc()

        # ---- S8: h1d = H[Nc,:]^T h1c (partial over Nc), chunked AR -------
        hrow_state = {}

        def hrow_moving(t, lo, w):
            chunk = lo // HALF
            slab_i = t // SLAB
            key = (chunk, slab_i)
            if key not in hrow_state:
                sb = stream.tile([128, SLAB * HALF], fp8, tag="slab",
                                 name=f"rj{chunk}")
                base = chunk * HT * HALF + slab_i * SLAB * HALF
                nc.scalar.dma_start(
                    out=sb[:], in_=hrow_d.ap()[:, base:base + SLAB * HALF])
                hrow_state[key] = sb
            return hrow_state[key][:, (t % SLAB) * HALF + (lo % HALF):
                                   (t % SLAB) * HALF + (lo % HALF) + w]

        h1dA, h1dB = chunked_bmm_ag_v(h1cv, hrow_moving, "cc2")

        # ---- S9+S10: hT = (De[Ec,:] h1d)^T / DSCALE + eps*hx -------------
        h1e_ps = [ps.tile([128, F], fp32, tag="ps", name=f"h1e{i}")
                  for i in range(HALF // F)]
        for j, ej in slab_stream(deT_d, fp8d, NT, "ej"):
            for blk in range(HALF // F):
                nc.tensor.matmul(h1e_ps[blk][:], vtile(h1dA, h1dB, j),
                                 ej[:, blk * F:(blk + 1) * F],
                                 start=(j == 0), stop=(j == NT - 1))
        hT = med.tile([D, HALF], fp32, tag="hxT")
        for blk in range(HALF // F):
            sl = slice(blk * F, (blk + 1) * F)
            nc.vector.scalar_tensor_tensor(hT[:, sl], h1e_ps[blk][:],
                                           1.0 / DSCALE, ehxT[:, sl],
                                           op0=Alu.mult, op1=Alu.add)
        hv = med.tile([128, HALF], bf16, tag="h1av")
        for t in range(HT):
            pt = ps.tile([128, 128], fp32, tag="ps")
            nc.tensor.transpose(pt[:], hT[:, t * 128:(t + 1) * 128], ident[:])
            nc.vector.tensor_copy(hv[:, t * 128:(t + 1) * 128], pt[:])
        warm_cc()

        # ---- S11: out = H h (partial over Ec), bf16 T-layout chunked AR --
        s_bn = small.tile([D, 1], fp32, tag="s_bn")
        nc.vector.tensor_scalar_add(s_bn[:], bnv_t[:], BN_EPS)
        nc.scalar.activation(s_bn[:], s_bn[:], Act.Sqrt)
        nc.vector.reciprocal(s_bn[:], s_bn[:])
        nc.vector.tensor_mul(s_bn[:], s_bn[:], bng_t[:])
        t_bn = small.tile([D, 1], fp32, tag="t_bn")
        nc.vector.tensor_mul(t_bn[:], bnm_t[:], s_bn[:])
        nc.vector.tensor_tensor(t_bn[:], bnb_t[:], t_bn[:], op=Alu.subtract)

        # two half-width ReduceScatters (core even gets summed cols 0:HALF,
        # odd the rest). S11's matmuls are grouped into quarter-pairs so
        # RS h=0 fires after only half the matmuls; its epilogue overlaps
        # the rest of S11 and RS h=1.
        Q = HALF // 2
        cc3_in = [dram.tile([256, Q], bf16, tag=f"cc3i{h}", name=f"cc3i{h}")
                  for h in range(2)]
        cc3_out = [dram.tile([128, Q], bf16, tag=f"cc3o{h}", name=f"cc3o{h}")
                   for h in range(2)]
        htr_s11 = make_qstate()
        for h in range(2):
            # blocks covering cols [h*Q:(h+1)*Q] of both n-chunks
            pss = [ps.tile([128, F], fp32, tag="ps", name=f"out_{h}_{i}")
                   for i in range(4)]
            for t in range(HT):
                for i in range(4):
                    chunk, blk = divmod(i, 2)
                    q = 2 * chunk + h
                    nc.tensor.matmul(
                        pss[i][:],
                        hv[:, t * 128:(t + 1) * 128],
                        qtile(htr_s11, t, q, "htq11")[:, blk * F:
                                                      (blk + 1) * F],
                        start=(t == 0), stop=(t == HT - 1))
            ccv = med.tile([128, HALF + ZPAD], bf16, tag=f"ccv{h}")
            for i in range(4):
                chunk, blk = divmod(i, 2)
                sl = slice(chunk * Q + blk * F, chunk * Q + (blk + 1) * F)
                nc.vector.tensor_copy(ccv[:, sl], pss[i][:])
            for chunk in range(2):
                nc.sync.dma_start(
                    out=cc3_in[h][chunk * 128:(chunk + 1) * 128, :],
                    in_=ccv[:, chunk * Q:(chunk + 1) * Q])
            nc.gpsimd.collective_compute(
                "ReduceScatter", Alu.add, replica_groups=PAIRS,
                ins=[cc3_in[h].opt()], outs=[cc3_out[h].opt()])
        for h in range(2):
            res3 = med.tile([128, HALF + ZPAD], bf16, tag=f"resv{h}")
            nc.sync.dma_start(out=res3[:, 0:Q], in_=cc3_out[h][:])
            of = med.tile([D, Q], fp32, tag=f"of{h}")
            nc.scalar.activation(of[:], res3[:, 0:Q], Act.Lrelu, alpha=0.01)
            nc.vector.tensor_scalar(of[:], of[:], s_bn[:], t_bn[:],
                                    op0=Alu.mult, op1=Alu.add)
            nc.sync.dma_start(out=y_d.ap()[:, h * Q:(h + 1) * Q], in_=of[:])

    nc.finalize()
    return nc


def _get_nc():
    if "nc" not in _CACHE:
        _CACHE["nc"] = _build()
    return _CACHE["nc"]


def _tile128(a):
    """[K*128, W] -> [128, K*W] block-transposed stream layout."""
    K = a.shape[0] // 128
    return np.ascontiguousarray(
        a.reshape(K, 128, a.shape[1]).transpose(1, 0, 2).reshape(
            128, K * a.shape[1]))


def _shard(inputs):
    import ml_dtypes
    bf16 = ml_dtypes.bfloat16
    fp8 = ml_dtypes.float8_e4m3
    fp8d = ml_dtypes.float8_e3m4

    H = np.asarray(inputs["incident_mat"], dtype=np.float32)
    Dv = np.asarray(inputs["degree_v"], dtype=np.float32)
    De = np.asarray(inputs["degree_e"], dtype=np.float32)
    x = np.asarray(inputs["x"], dtype=np.float32)
    em = np.asarray(inputs["e_masks"])
    w = np.asarray(inputs["mlp_W"], dtype=np.float32)
    b = np.asarray(inputs["mlp_b"], dtype=np.float32)
    th = np.asarray(inputs["theta_att"], dtype=np.float32).reshape(D)
    eps = np.full((D, 1), float(np.asarray(inputs["eps"]).reshape(-1)[0]),
                  dtype=np.float32)

    def col(v):
        return np.ascontiguousarray(
            np.asarray(v, dtype=np.float32).reshape(D, 1))

    bng, bnb = col(inputs["bn_gamma"]), col(inputs["bn_beta"])
    bnm, bnv = col(inputs["bn_mean"]), col(inputs["bn_var"])

    whi = w.astype(bf16)
    th2 = np.ascontiguousarray(th.reshape(D, 1))
    brow = np.ascontiguousarray(b.reshape(1, D))

    in_maps = []
    for g in range(B):
        Hg8 = H[g].astype(fp8)
        HgT8 = np.ascontiguousarray(H[g].T).astype(fp8)
        dege_full = H[g].sum(axis=0, dtype=np.float32)
        xv = _tile128(x[g]).astype(bf16)
        DvT = (Dv[g].T * DSCALE).astype(fp8d)
        DeT = (De[g].T * DSCALE).astype(fp8d)
        for c in range(2):
            lo, hi = c * HALF, (c + 1) * HALF
            # hrow chunk-major: [128, 2*HT*HALF], chunk c2 at t*HALF+e
            hrow_g = Hg8[lo:hi, :].reshape(HT, 128, 2, HALF)
            hrow_t = np.ascontiguousarray(
                hrow_g.transpose(1, 2, 0, 3).reshape(128, 2 * HT * HALF))
            htr_q = np.ascontiguousarray(
                HgT8[lo:hi, :].reshape(HT, 128, 4, HALF // 2)
                .transpose(1, 2, 0, 3).reshape(128, 2 * HT * HALF))
            in_maps.append({
                "xv": xv,
                "hcol": _tile128(np.ascontiguousarray(Hg8[:, lo:hi])),
                "htr": htr_q,
                "hrow": hrow_t,
                "dvT": _tile128(np.ascontiguousarray(DvT[:, lo:hi])),
                "deT": _tile128(np.ascontiguousarray(DeT[:, lo:hi])),
                "dege": np.ascontiguousarray(
                    dege_full[lo:hi].reshape(1, HALF)),
                "whi": whi, "th2": th2, "brow": brow,
                "mask": np.ascontiguousarray(
                    em[g, lo:hi].astype(np.float32).reshape(1, HALF)),
                "eps": eps,
                "bng": bng, "bnb": bnb, "bnm": bnm, "bnv": bnv,
            })
    return in_maps


def kernel(**inputs):
    from concourse.bass_utils import run_bass_kernel_spmd

    nc = _get_nc()
    in_maps = _shard(inputs)
    res = run_bass_kernel_spmd(nc, in_maps, list(range(NCORES)))
    out = np.empty((B, N, D), dtype=np.float32)
    for g in range(B):
        out[g, :HALF, :] = res.results[2 * g]["y"].T
        out[g, HALF:, :] = res.results[2 * g + 1]["y"].T
    return out
